# revision 10
# baseline (speedup 1.0000x reference)
"""ExtraMSAEmbedding Trainium2 kernel (v3 — all-bf16, quadrant-parallel).

out[s, r, :] = one_hot(msa[s, r], 23) @ W[:, :23].T
             + has_del[s, r] * W[:, 23] + del_val[s, r] * W[:, 24] + b

The harness gate is rel_err < 2e-2 against max|out| (=1.37), so the whole
pipeline runs in bf16 (measured end-to-end rel err 5.1e-3): bf16 weights
and deletion feats, exact bf16 one-hot, f32 PSUM accumulation, bf16
output.  That halves the dominant HBM write traffic (12.6 MB/core) and
cuts matmul work 3x vs an exact-fp32 3-pass decomposition.

Strategy (8 NeuronCores, data-parallel over the 2048 sequences — 256 seqs
= 98304 tokens per core, blocks of 512 tokens, 4 blocks per iteration,
8 iterations per super-block, 6 super-blocks):

- PE tile placement: full-width (M=128) matmuls serialize on the PE
  columns and expose each instruction's ~300ns pipeline fill (v2 measured
  95.8% TensorMatrix busy, 601ns/matmul, zero overlap).  v3 uses six
  column/row-disjoint 32x64 / 32-row tile slots per iteration so fills
  overlap: 4 main matmuls (K=26, M=64) at (32g, 64*(g//2)) and 2
  broadcast matmuls (K=4, M=64) at (64,0) / (0,64).
- broadcast: msa arrives bf16 [4, 4096] per super, staged twice (SBUF
  partitions 64-67 for bc_A -> pb[0:64], partitions 0-3 for bc_B ->
  pb[64:128]); block-diagonal 0/1 lhsT replicates each group's msa row
  onto its 32 PSUM partitions (class rows; rows 23+ get 0.0).
- one DVE tensor_scalar(is_equal) per TWO iterations (FD=1024, 2 PSUM
  banks) vs a per-partition class column -> transposed one-hot
  [class, token] in bf16.  Row 25 compares 0==0 -> 1.0 (bias row); rows
  23/24 are overwritten by a per-super DMA of has_del/del_val (bf16).
- mains: single bf16 pass, K=26, stationary [W.T(23); w23; w24; b].
- PSUM -> SBUF bf16 cast copies split ACT/DVE to balance (measured
  1136ns vs 1220ns per [128,1024] copy; DVE also carries the eq ops):
  37 ACT / 11 DVE out of 48.
- outputs leave as raw [super, 128, iter, 1024] bf16 dumps via SWDGE
  (descriptors spread over all 16 SDMA engines); the host does the final
  cheap layout transpose + f32 cast while unsharding.
"""

import numpy as np

N_SEQ, N_RES = 2048, 384
C_OUT = 64
N_CORES = 8
SEQ_PER_CORE = N_SEQ // N_CORES  # 256
T_PER_CORE = SEQ_PER_CORE * N_RES  # 98304
BLK = 512  # tokens per block (one PSUM bank of f32)
N_BLOCKS = T_PER_CORE // BLK  # 192
GROUPS = 4  # blocks per iteration
SUPER = 8  # iterations per super-block

_CACHE: dict = {}
_LAST_RESULT = None


def build_program(n_blocks: int = N_BLOCKS):
    """Build + compile the Bass/Tile program (same program for all cores)."""
    import concourse.bass as bass  # noqa: F401
    import concourse.mybir as mybir
    import concourse.tile as tile
    from concourse import bacc

    f32 = mybir.dt.float32
    bf16 = mybir.dt.bfloat16
    assert n_blocks % (GROUPS * SUPER) == 0
    n_super = n_blocks // (GROUPS * SUPER)
    FREE = SUPER * BLK  # free-dim of the big per-super tiles

    nc = bacc.Bacc("TRN2", target_bir_lowering=False, debug=False)

    # inputs laid out per super-block by the host (see kernel() below)
    # msa bf16 (exact for ints 0..22): partition g holds group g's tokens
    msa_d = nc.dram_tensor(
        "msa", [n_super, GROUPS, SUPER, BLK], bf16, kind="ExternalInput"
    ).ap()
    # has_del / del_val, one bf16 plane each -> feat rows 23, 24
    hd_d = nc.dram_tensor(
        "hd", [n_super, 2, GROUPS, SUPER, BLK], bf16, kind="ExternalInput"
    ).ap()
    # stationary weights: four identical K=32 strips of [W.T; w23; w24; b]
    w128_d = nc.dram_tensor("w128", [128, C_OUT], bf16, kind="ExternalInput").ap()
    # broadcast masks: rows 64-67 select groups 0/1 (bc_A), rows 0-3
    # select groups 2/3 (bc_B)
    bmask_d = nc.dram_tensor("bmask", [128, C_OUT], bf16, kind="ExternalInput").ap()
    ccol_d = nc.dram_tensor("ccol", [128, 1], f32, kind="ExternalInput").ap()
    # raw output dump: [super, 128 partitions, SUPER iters, 1024] bf16 ->
    # per partition each half-super store is one contiguous 8 KB run
    out_d = nc.dram_tensor(
        "out", [n_super, 128, SUPER, 2 * BLK], bf16, kind="ExternalOutput"
    ).ap()

    with tile.TileContext(nc) as tc:
        with (
            tc.tile_pool(name="staging", bufs=3) as spool,
            tc.tile_pool(name="feat", bufs=3) as fpool,
            tc.tile_pool(name="osb", bufs=3) as opool,
            tc.tile_pool(name="consts", bufs=1) as cpool,
            tc.tile_pool(name="pbc", bufs=2, space=bass.MemorySpace.PSUM) as pbpool,
            tc.tile_pool(name="pout", bufs=2, space=bass.MemorySpace.PSUM) as popool,
        ):
            # const loads on the Scalar HWDGE ring so the first msa staging
            # DMA isn't queued behind them on Sync
            w128 = cpool.tile([128, C_OUT], bf16)
            nc.scalar.dma_start(w128[:], w128_d)
            bmask = cpool.tile([128, C_OUT], bf16)
            nc.scalar.dma_start(bmask[:], bmask_d)
            ccol = cpool.tile([128, 1], f32)
            nc.scalar.dma_start(ccol[:], ccol_d)

            ndve = 0  # DVE copy counter (balance 11 DVE / 37 ACT)
            for s in range(n_super):
                # per-super msa staging, two copies: partitions 64-67 feed
                # bc_A (PE rows 64-67), partitions 0-3 feed bc_B
                staging = spool.tile([128, FREE], bf16)
                nc.sync.dma_start(staging[64:68, :], msa_d[s])
                nc.sync.dma_start(staging[0:4, :], msa_d[s])

                feat = fpool.tile([128, FREE], bf16)
                for jj in range(SUPER // 2):  # iteration pairs
                    pb = pbpool.tile([128, 2 * BLK], f32, name="pb")
                    for j2 in range(2):
                        ps = slice(j2 * BLK, (j2 + 1) * BLK)
                        cs = slice((2 * jj + j2) * BLK, (2 * jj + j2 + 1) * BLK)
                        # bc_A: pb[32g+k, t] = msa_g[t] for g=0,1 k<23
                        nc.tensor.matmul(
                            pb[0:64, ps],
                            bmask[64:68, :],
                            staging[64:68, cs],
                            tile_position=(64, 0),
                        )
                        # bc_B: same for groups 2,3
                        nc.tensor.matmul(
                            pb[64:128, ps],
                            bmask[0:4, :],
                            staging[0:4, cs],
                            tile_position=(0, 64),
                        )
                    # one-hot (+ ones row 25) via is_equal vs class column,
                    # two blocks-of-4 per instruction (FD=1024)
                    ecs = slice(2 * jj * BLK, (2 * jj + 2) * BLK)
                    nc.vector.tensor_scalar(
                        feat[:, ecs], pb[:], ccol[:], None, mybir.AluOpType.is_equal
                    )

                # deletion features (bf16) into rows 23, 24 of each 32-row
                # group (after the eq ops in program order; Tile serializes
                # the overlapping writes).  On the otherwise-idle Sync ring.
                for k in range(2):
                    nc.sync.dma_start(feat[23 + k : 128 : 32, :], hd_d[s, k])

                # osb layout per partition: [iter j | bank | 512 tokens]
                osb = opool.tile([128, SUPER * 2 * BLK], bf16, name="osb")
                for j in range(SUPER):
                    cs = slice(j * BLK, (j + 1) * BLK)
                    po = popool.tile([128, 2 * BLK], f32, name="po")
                    # main matmuls: po[64-chunk] = w26.T @ feat_g, K=26,
                    # M=64, single bf16 pass, quadrant slot (32g, 64*(g//2))
                    for g in range(GROUPS):
                        bank, half = g % 2, 64 * (g // 2)
                        nc.tensor.matmul(
                            po[half : half + 64, bank * BLK : (bank + 1) * BLK],
                            w128[32 * g : 32 * g + 26, :],
                            feat[32 * g : 32 * g + 26, cs],
                            tile_position=(32 * g, half),
                        )
                    # PSUM -> SBUF bf16 cast: 37 ACT / 11 DVE
                    ocs = slice(j * 2 * BLK, (j + 1) * 2 * BLK)
                    if j % 4 == 3 and ndve < 11:
                        ndve += 1
                        nc.vector.tensor_copy(osb[:, ocs], po[:])
                    else:
                        nc.scalar.copy(osb[:, ocs], po[:])
                    # raw store via SWDGE (descriptors spread over all 16
                    # SDMA engines), half a super-block at a time
                    if j % (SUPER // 2) == SUPER // 2 - 1:
                        h2 = j // (SUPER // 2)
                        hs = slice(h2 * (SUPER // 2), (h2 + 1) * (SUPER // 2))
                        nc.gpsimd.dma_start(
                            out_d[s, :, hs, :],
                            osb[:, h2 * FREE : h2 * FREE + FREE],
                        )

    nc.compile()
    return nc


def _host_constants(W: np.ndarray, b: np.ndarray):
    import ml_dtypes

    bf = ml_dtypes.bfloat16
    f32 = np.float32
    # K=32 strip: [W.T classes(23); w23; w24; b; 0...], replicated 4x
    w26 = np.zeros((32, C_OUT), f32)
    w26[0:23] = W.T[0:23].astype(f32)
    w26[23] = W.T[23].astype(f32)
    w26[24] = W.T[24].astype(f32)
    w26[25] = b.astype(f32)
    w128 = np.tile(w26, (4, 1)).astype(bf)

    bmask = np.zeros((128, C_OUT), bf)
    for k in range(2):  # bc_A rows 64-67: groups 0,1 in cols 0-63
        bmask[64 + k, 32 * k : 32 * k + 23] = 1.0
    for k in range(2):  # bc_B rows 0-3 (rows 2,3 used): groups 2,3
        bmask[2 + k, 32 * k : 32 * k + 23] = 1.0

    ccol = np.full((128, 1), -7.0, f32)
    for p in range(128):
        j = p % 32
        if j < 23:
            ccol[p] = j  # one-hot compare value
        elif j == 25:
            ccol[p] = 0.0  # matches the broadcast 0 -> constant 1.0 (bias)
    return w128, bmask, ccol


def _stage_blocks(x_blocks: np.ndarray) -> np.ndarray:
    """[n_blocks, BLK] -> [n_super, GROUPS, SUPER, BLK] staging layout.

    Element [s, g, j] = flat block 4*(SUPER*s + j) + g.
    """
    nb = x_blocks.shape[0]
    x = x_blocks.reshape(nb // (GROUPS * SUPER), SUPER, GROUPS, BLK)
    return np.ascontiguousarray(x.transpose(0, 2, 1, 3))


def kernel(extra_msa, extra_has_deletion, extra_deletion_value, W, b):
    from concourse.bass_utils import run_bass_kernel_spmd

    import ml_dtypes

    bf = ml_dtypes.bfloat16
    f32 = np.float32
    msa = np.asarray(extra_msa).astype(f32)  # int -> f32 (exact for 0..22)
    has_ = np.asarray(extra_has_deletion, dtype=f32).astype(bf)
    del_ = np.asarray(extra_deletion_value, dtype=f32).astype(bf)
    W = np.asarray(W, dtype=f32)
    b = np.asarray(b, dtype=f32)

    if "nc" not in _CACHE:
        _CACHE["nc"] = build_program(N_BLOCKS)
    nc = _CACHE["nc"]

    w128, bmask, ccol = _host_constants(W, b)

    in_maps = []
    for c in range(N_CORES):
        s0, s1 = c * SEQ_PER_CORE, (c + 1) * SEQ_PER_CORE
        hd = np.stack(
            [
                _stage_blocks(np.ascontiguousarray(x[s0:s1]).reshape(N_BLOCKS, BLK))
                for x in (has_, del_)
            ],
            axis=1,  # [n_super, 2, GROUPS, SUPER, BLK]
        )
        in_maps.append(
            {
                "msa": _stage_blocks(msa[s0:s1].reshape(N_BLOCKS, BLK)).astype(bf),
                "hd": hd,
                "w128": w128,
                "bmask": bmask,
                "ccol": ccol,
            }
        )

    res = run_bass_kernel_spmd(nc, in_maps, list(range(N_CORES)))
    global _LAST_RESULT
    _LAST_RESULT = res

    # unshard: raw [super, 128, SUPER, 1024] bf16 -> token-major f32
    # p = phalf*64 + ch, f = bank*512 + t, block = 4*(8s+j) + 2*phalf + bank
    n_super = N_BLOCKS // (GROUPS * SUPER)
    parts = []
    for r in res.results:
        raw = np.asarray(r["out"]).reshape(n_super, 2, C_OUT, SUPER, 2, BLK)
        tok = raw.transpose(0, 3, 1, 4, 5, 2).reshape(T_PER_CORE, C_OUT)
        parts.append(tok.astype(f32).reshape(SEQ_PER_CORE, N_RES, C_OUT))
    return np.ascontiguousarray(np.concatenate(parts, axis=0))
